# revision 58
# baseline (speedup 1.0000x reference)
"""Trainium2 Bass kernel for nn_MultiHeadAttention_36223754174786.

Fused transformer block: QKV projection -> 16-head attention (naive, full
[S,S] scores) -> LayerNorm -> FeedForward (relu MLP) with residual.
B=2, S=2048, D=1024, H=16, DK=64, FF_HIDDEN=2048.

Sharding: tensor-parallel over heads, all 8 cores in one group.  Core c
computes heads {2c, 2c+1} for BOTH batches (QKV projection columns sharded
8-way -> zero replicated projection FLOPs), then an 8-core AllToAll
redistributes attention outputs so core c ends up with the full 16-head
attention for its own 512 tokens (batch c//4, rows 512*(c%4)..) on which it
runs LayerNorm + FFN + residual (token-sharded FFN, no second collective).

Cost-model facts this layout exploits (instruction_cost_v2.rs):
  - matmul time = out free-size * cycles_per_row, independent of K and M.
    cycles_per_row keys on the MOVING operand (ins[0]=rhs): bf16 is 1.0
    always; f32r is 4.0 when out free < 256.  Every matmul keeps its rhs
    bf16, and attn@V is emitted "flipped" (out[q,65] = expT.T @ [V_h|1],
    M=128 full, ap=65) which halves its cost vs the [65,q] orientation and
    lands token-major (no PE transpose of the attention output).
  - a start=True matmul wipes accumulation pending-state for its whole
    PSUM bank, so the 8 interleaved attn@V groups sharing 2 banks are
    preceded by one K=1 zeroing matmul and accumulate with start=False.
  - AllToAll costs 15us + bytes/40GB/s on a separate COLLECTIVE_CORES
    resource.  Queries are processed in 2 stages (token halves, host-side
    permuted so each stage window is contiguous) so stage 0's exchange
    overlaps stage 1's attention compute and the FFN of stage 0 overlaps
    stage 1's exchange.
  - All DMAs ride the sync(SP) HWDGE queue or the gpsimd SWDGE queue,
    never the scalar queue: a DMA dispatched from the scalar queue holds
    the ACT sequencer ~1us and stalls the exp stream.  Queues execute in
    order, so the FFN weight loads are emitted after every x chunk
    (cannot steal serialized DMA-engine bandwidth from the projections),
    and the latency-critical A2A staging/gather DMAs go on gpsimd.
  - ACT exp of [128,1024] costs ~1.04us vs ~0.66us of PE work per key
    chunk, so 1/4 of the chunks exp on the DVE via a Schraudolph int16
    bitcast (see below) and the scores psum pool is 3 tiles deep for the
    phases that no longer share banks with the projection accumulators.

Numerics: all matmul operands bf16 (accumulation f32 in PSUM); probs are
rounded to bf16 before attn@V; attention crosses the collective in bf16;
1/4 of probs carry the ~2% RMS fast-exp error.  LayerNorm renormalizes
scale, so relative operand error passes through.  Measured 1.14e-2 vs the
2e-2 gate.

Timeline-sim 257.2us vs 353.5us for the data-parallel baseline; PE floor
~435k cycles (~181us): proj q/k 65k + v 33k + scores 131k + attnV 67k +
ffiT transpose 8k + FFN 131k.  Remaining known slack: ~18us A2A#0-tail+LN
gap before ffn0 (no independent PE work exists to fill it), ~10us of
per-instruction sem overheads in attention, ~6us program drain.
"""

import numpy as np

import concourse.bass as bass
import concourse.tile as tile
from concourse import bacc, mybir
from concourse.bass_utils import run_bass_kernel_spmd
from concourse.masks import make_identity

F32 = mybir.dt.float32
F32R = mybir.dt.float32r
I16 = mybir.dt.int16
BF16 = mybir.dt.bfloat16
AF = mybir.ActivationFunctionType
OP = mybir.AluOpType

# Schraudolph fast-exp in bf16: exp(s) ~= bitcast<bf16>(int16(s*EXP_A+EXP_B))
# (bf16 shares f32's 8 exponent bits so the classic trick works at 2^7
# mantissa scale).  ~2% RMS probability error; softmax renormalization
# cancels most of it.  Lets the DVE share the exp load with ACT so the
# attention phases stay PE-bound.  bf16 keeps the matmul verifier-legal
# (f32/f32r operands must match on both sides).
EXP_A = float(np.float32(2 ** 7 / np.log(2)))
EXP_B = float(np.float32(127 * 128 - 366393 / 65536.0))

B, S, D, H = 2, 2048, 1024, 16
DK = D // H          # 64
FF = 2048
P = 128
N_CORES = 8
KC = S // P          # 16 key chunks
DCH = D // P         # 8 chunks of the model dim
FFC = FF // P        # 16 chunks of the ffn hidden dim
NST = 2              # A2A stages (token halves)
QW = S // NST        # query window per (stage, batch): 1024
TOK = 512            # owned tokens per core
REP = [list(range(N_CORES))]
PHASE_MARKS = []


def _bcast_ap(ap):
    """Partition-broadcast a 1-D DRAM vector to [128, n] for DMA."""
    return bass.AP(tensor=ap.tensor, offset=ap.offset, ap=[[0, P]] + list(ap.ap))


def build_program(ln_affine=True, b2_zero=False):
    nc = bacc.Bacc("TRN2", target_bir_lowering=False, debug=False,
                   num_devices=N_CORES)

    def mm(out_ap, lhsT, rhs, start, stop):
        nc.tensor.matmul(out_ap, lhsT, rhs, start=start, stop=stop)

    # x transposed host-side; xq token-permuted (stage-major, see kernel()).
    xqT = nc.dram_tensor("xqT", [D, B * S], BF16, kind="ExternalInput")
    xkT = nc.dram_tensor("xkT", [D, B * S], BF16, kind="ExternalInput")
    xvT = nc.dram_tensor("xvT", [D, B * S], BF16, kind="ExternalInput")
    wq = nc.dram_tensor("wq", [D, P], BF16, kind="ExternalInput")  # my cols
    wk = nc.dram_tensor("wk", [D, P], BF16, kind="ExternalInput")
    wv = nc.dram_tensor("wv", [D, P], BF16, kind="ExternalInput")
    w1 = nc.dram_tensor("w1", [D, FF], BF16, kind="ExternalInput")
    w2 = nc.dram_tensor("w2", [FF, D], BF16, kind="ExternalInput")
    bq = nc.dram_tensor("bq", [P], F32, kind="ExternalInput")
    bk = nc.dram_tensor("bk", [P], F32, kind="ExternalInput")
    bv = nc.dram_tensor("bv", [P], F32, kind="ExternalInput")
    b1 = nc.dram_tensor("b1", [FF], F32, kind="ExternalInput")
    b2 = nc.dram_tensor("b2", [D], F32, kind="ExternalInput")
    ln_g = nc.dram_tensor("ln_g", [D], F32, kind="ExternalInput")
    ln_b = nc.dram_tensor("ln_b", [D], F32, kind="ExternalInput")
    out = nc.dram_tensor("out", [TOK, D], F32, kind="ExternalOutput")


    with tile.TileContext(nc) as tc:
        with (
            tc.tile_pool(name="const", bufs=1) as cp,
            tc.tile_pool(name="dram", bufs=1, space="DRAM") as dram,
            tc.tile_pool(name="qkv", bufs=1) as qp,
            tc.tile_pool(name="fw", bufs=1) as fw,
            tc.tile_pool(name="xs", bufs=14) as xs,
            tc.tile_pool(name="aE", bufs=3) as aE,
            tc.tile_pool(name="aSt", bufs=2) as aSt,
            tc.tile_pool(name="lnp", bufs=4) as lnp,
            tc.tile_pool(name="fp", bufs=1) as fp,
        ):
            ident = cp.tile([P, P], F32, tag="ident", name="ident")
            make_identity(nc, ident)
            # zero operands for the psum-bank-clearing matmul (a start=True
            # group wipes accumulation pending-state for its whole bank, so
            # banks holding several interleaved groups are zeroed once by a
            # K=1 matmul and every real group accumulates with start=False)
            zl = cp.tile([1, P], BF16, tag="zl", name="zl")
            nc.vector.memset(zl, 0.0)
            zr = cp.tile([1, 512], BF16, tag="zr", name="zr")
            nc.vector.memset(zr, 0.0)
            eps_t = cp.tile([P, 1], F32, tag="eps", name="eps")
            nc.vector.memset(eps_t, 1e-5)
            ones_t = cp.tile([P, 2, 1], F32, tag="ones", name="ones")
            nc.vector.memset(ones_t, 1.0)
            bq_col = cp.tile([P, 1], F32, tag="bqc", name="bqc")
            bk_col = cp.tile([P, 1], F32, tag="bkc", name="bkc")
            b1_col = cp.tile([P, FFC], F32, tag="b1c", name="b1c")
            bv_b = cp.tile([P, P], F32, tag="bvb", name="bvb")
            nc.sync.dma_start(bq_col, bq[:].rearrange("(o p) -> p o", p=P))
            nc.sync.dma_start(bk_col, bk[:].rearrange("(o p) -> p o", p=P))
            nc.sync.dma_start(b1_col, b1[:].rearrange("(o p) -> p o", p=P))
            nc.gpsimd.dma_start(bv_b, _bcast_ap(bv[:]))
            if ln_affine:
                lng_b = cp.tile([P, D], F32, tag="lng", name="lng")
                lnb_b = cp.tile([P, D], F32, tag="lnb", name="lnb")
                nc.gpsimd.dma_start(lng_b, _bcast_ap(ln_g[:]))
                nc.gpsimd.dma_start(lnb_b, _bcast_ap(ln_b[:]))
            if not b2_zero:
                b2_b = cp.tile([P, D], F32, tag="b2b", name="b2b")
                nc.gpsimd.dma_start(b2_b, _bcast_ap(b2[:]))

            # my projection weight columns: [128 part (contraction rows),
            # 8 chunks, 128 cols]
            wqkv_sb = {}
            for which, wsrc in (("k", wk), ("v", wv), ("q", wq)):
                wt = cp.tile([P, DCH, P], BF16, tag=f"w_{which}",
                             name=f"w_{which}")
                nc.gpsimd.dma_start(
                    wt, wsrc[:, :].rearrange("(c p) n -> p c n", p=P))
                wqkv_sb[which] = wt

            # FFN weight tiles; DMAs are emitted later (after the stage-0
            # attention) so they don't hog the DMA engines while the x
            # activation streams gate the projection start.
            w1_sb = [fw.tile([P, FF], BF16, tag=f"w1_{k}", name=f"w1_{k}")
                     for k in range(DCH)]
            w2_sb = [fw.tile([P, D], BF16, tag=f"w2_{k}", name=f"w2_{k}")
                     for k in range(FFC)]

            def load_ffn_weights():
                # sync (SP) queue only, POSITIONED after every x-chunk DMA:
                # queues run in order so these can't steal DMA bandwidth from
                # the x streams, and the SP semaphore is not shared with any
                # compute engine (a late DMA on the scalar queue would delay
                # every wait on the Activation semaphore issued after it).
                for k in range(DCH):
                    nc.sync.dma_start(w1_sb[k], w1[k * P:(k + 1) * P, :])
                for k in range(FFC):
                    nc.sync.dma_start(w2_sb[k], w2[k * P:(k + 1) * P, :])

            # A2A bounce buffers, one pair per stage.
            # layout [dest|src 8, sub 2, tok 128, col 128]
            inb = [dram.tile([N_CORES, 2, P, P], BF16, tag=f"inb{s}",
                             name=f"inb{s}") for s in range(NST)]
            outb = [dram.tile([N_CORES, 2, P, P], BF16, tag=f"outb{s}",
                              name=f"outb{s}") for s in range(NST)]

            # persistent per-batch activation tiles
            qT = [qp.tile([P, S], BF16, tag=f"qT{b}", name=f"qT{b}")
                  for b in range(B)]
            kT = [qp.tile([P, S], BF16, tag=f"kT{b}", name=f"kT{b}")
                  for b in range(B)]
            v_sb = [[qp.tile([P, 2, DK + 1], BF16, tag=f"v{b}_{t}",
                             name=f"v{b}_{t}") for t in range(KC)]
                    for b in range(B)]

            def emit_proj(b, acc, mid=None):
                """Project q, k, v columns of batch b (my 128 head-pair cols).

                qT/kT: [128 pair-dims, 2048 tok]; v_sb: key-major
                [128 keys, 2 heads, 65] with a ones column in col 64.
                `mid` is emitted after the (q, half 0) group: the stage-0
                attention of this batch only needs k/v plus the first q
                window, so it starts while the last 2MB of x still streams."""
                HS = S // 2  # half-token granularity keeps x residency small
                for which, hf in (("k", 0), ("k", 1), ("v", 0), ("v", 1),
                                  ("q", 0), ("mid", 0), ("q", 1)):
                    if which == "mid":
                        if mid is not None:
                            mid()
                        continue
                    xsrc = {"k": xkT, "v": xvT, "q": xqT}[which]
                    wt = wqkv_sb[which]
                    if True:
                        c0 = b * S + hf * HS
                        x_sb = []
                        for k in range(DCH):
                            xt = xs.tile([P, HS], BF16, tag="x",
                                         name=f"x{which}{b}{hf}_{k}")
                            nc.sync.dma_start(
                                xt, xsrc[k * P:(k + 1) * P, c0:c0 + HS])
                            x_sb.append(xt)
                        # psum -> SBUF copies on DVE (gpsimd cannot touch
                        # PSUM; ACT is near-saturated with exps in the
                        # attention phase this overlaps)
                        if which in ("q", "k"):
                            dst = qT[b] if which == "q" else kT[b]
                            bias = bq_col if which == "q" else bk_col
                            for t4 in range(HS // 512):
                                ps = acc.tile([P, 512], F32, tag="acc",
                                              name="acc")
                                for k in range(DCH):
                                    mm(ps, wt[:, k, :],
                                       x_sb[k][:, t4 * 512:(t4 + 1) * 512],
                                       start=(k == 0), stop=(k == DCH - 1))
                                nc.vector.tensor_scalar_add(
                                    dst[:, hf * HS + t4 * 512:
                                        hf * HS + (t4 + 1) * 512], ps, bias)
                        else:
                            for tg in range(HS // 512):
                                ps = acc.tile([P, 512], F32, tag="acc",
                                              name="acc")
                                for i in range(4):
                                    t = hf * (HS // P) + tg * 4 + i
                                    for k in range(DCH):
                                        mm(ps[:, i * P:(i + 1) * P],
                                           x_sb[k][:, (tg * 4 + i) * P:
                                                   (tg * 4 + i + 1) * P],
                                           wt[:, k, :],
                                           start=(k == 0), stop=(k == DCH - 1))
                                for i in range(4):
                                    t = hf * (HS // P) + tg * 4 + i
                                    nc.vector.tensor_tensor(
                                        v_sb[b][t][:, :, 0:DK],
                                        ps[:, i * P:(i + 1) * P].rearrange(
                                            "p (h d) -> p h d", h=2),
                                        bv_b[:].rearrange("p (h d) -> p h d",
                                                          h=2),
                                        OP.add)
                                    nc.vector.tensor_copy(
                                        v_sb[b][t][:, :, DK:DK + 1], ones_t)

            def emit_attn(st, b, psS, psA):
                """Attention for stage window st of batch b, both my heads.

                scores sT [keys, q] -> exp (bf16) -> flipped attn@V:
                out[q 128, 65] = expT.T @ [V_h|1]; denominator in col 64.
                Normalized output staged bf16 into the stage's A2A bounce."""
                q0 = st * QW
                stg = aSt.tile([P, 4, 2, P], BF16, tag=f"stg{b % 2}",
                               name=f"stg{st}_{b}")
                for h in range(2):
                    lo, hi = h * DK, (h + 1) * DK
                    pa = [psA.tile([P, 4, DK + 1], F32, tag="pa", name="pa")
                          for _ in range(2)]
                    for t_ in pa:
                        mm(t_[:].rearrange("p a b -> p (a b)"), zl,
                           zr[:, 0:4 * (DK + 1)], start=True, stop=True)
                    # software pipeline: attn@V for chunk kc-2 is emitted
                    # after the scores matmuls of chunk kc, so exp(kc) has a
                    # full chunk-period of slack before its attn@V consumers
                    # reach the PE queue head.  One [128, QW] exp per chunk:
                    # ACT then paces at ~1.04us/kc, just under the PE's
                    # per-chunk work, so the phase stays PE-bound with exact
                    # exp everywhere.
                    elag = [None] * KC

                    def attn_v(kc):
                        for sub in range(QW // P):
                            mm(pa[sub // 4][:, sub % 4, :],
                               elag[kc][:, sub * P:(sub + 1) * P],
                               v_sb[b][kc][:, h, :],
                               start=False, stop=(kc == KC - 1))

                    for kc in range(KC):
                        ps = psS.tile([P, QW], F32, tag="psS", name="psS")
                        for j in range(2):
                            mm(ps[:, j * 512:(j + 1) * 512],
                               kT[b][lo:hi, kc * P:(kc + 1) * P],
                               qT[b][lo:hi, q0 + j * 512:q0 + (j + 1) * 512],
                               start=True, stop=True)
                        if kc % 4 == 2:
                            # 1/4 of chunks: DVE fast-exp so the ACT exp
                            # stream (1.04us per chunk vs 0.66us of PE work)
                            # stops pacing the phase
                            ei = aE.tile([P, QW], I16, tag="expi", bufs=2,
                                         name="expi")
                            nc.vector.tensor_scalar(ei, ps, EXP_A, EXP_B,
                                                    OP.mult, OP.add)
                            elag[kc] = ei[:].bitcast(BF16)
                        else:
                            e = aE.tile([P, QW], BF16, tag="exp", bufs=5,
                                        name="exp")
                            nc.scalar.activation(e, ps, AF.Exp)
                            elag[kc] = e
                        if kc > 2:
                            attn_v(kc - 3)
                    attn_v(KC - 3)
                    attn_v(KC - 2)
                    attn_v(KC - 1)
                    rc = aSt.tile([P, 8], F32, tag=f"rc{b % 2}", name="rc")
                    for sub in range(QW // P):
                        pr = pa[sub // 4][:, sub % 4, :]
                        nc.vector.reciprocal(rc[:, sub:sub + 1],
                                             pr[:, DK:DK + 1])
                        nc.vector.tensor_scalar_mul(
                            stg[:, sub // 2, sub % 2, lo:hi],
                            pr[:, 0:DK], rc[:, sub:sub + 1])
                    # stage this head's half-columns into the bounce as
                    # soon as its normalize lands: the collective then only
                    # waits on the last head's sliver
                    dst = inb[st][b * 4:(b + 1) * 4, :, :, lo:hi].rearrange(
                        "d s t c -> t d s c")
                    nc.gpsimd.dma_start(dst, stg[:, :, :, lo:hi])

            def emit_ln(st, psTr):
                """Gather + LayerNorm + ffiT transposes for my stage-st
                token half."""
                ffi = [fp.tile([P, D], F32, tag=f"ffi{st}_{s}",
                               name=f"ffi{st}_{s}") for s in range(2)]
                ffiT = [fp.tile([P, 256], BF16, tag=f"ffiT{k}", bufs=2,
                                name=f"ffiT{k}") for k in range(DCH)]
                afs = []
                for sub in range(2):
                    af = lnp.tile([P, N_CORES, P], BF16, tag="af", name="af")
                    # gpsimd queue: DMAs dispatched from the scalar queue
                    # occupy the ACT sequencer (~1us each) and would stall
                    # exp dispatch; the sync queue's pending out-DMAs would
                    # delay this gather (it gates the LN)
                    nc.gpsimd.dma_start(
                        af, outb[st][:, sub, :, :].rearrange("s t c -> t s c"))
                    afs.append(af)
                for sub in range(2):
                    af = afs[sub]
                    afv = af[:].rearrange("p a b -> p (a b)")
                    stats = lnp.tile([P, 2, 6], F32, tag="stats", name="stats")
                    for sg in range(2):
                        nc.vector.bn_stats(stats[:, sg, :],
                                           afv[:, sg * 512:(sg + 1) * 512])
                    mv = lnp.tile([P, 2], F32, tag="mv", name="mv")
                    nc.vector.bn_aggr(mv, stats)
                    std = lnp.tile([P, 1], F32, tag="std", name="std")
                    nc.scalar.activation(std, mv[:, 1:2], AF.Sqrt, bias=eps_t)
                    rstd = lnp.tile([P, 1], F32, tag="rstd", name="rstd")
                    nc.vector.reciprocal(rstd, std)
                    nc.vector.tensor_scalar(ffi[sub], afv, mv[:, 0:1], rstd,
                                            OP.subtract, OP.mult)
                    if ln_affine:
                        nc.vector.tensor_mul(ffi[sub], ffi[sub], lng_b)
                        nc.vector.tensor_add(ffi[sub], ffi[sub], lnb_b)
                    for k in range(DCH):
                        pt = psTr.tile([P, P], F32, tag="ptr", name="ptr")
                        nc.tensor.transpose(pt, ffi[sub][:, k * P:(k + 1) * P],
                                            ident)
                        nc.vector.tensor_copy(
                            ffiT[k][:, sub * P:(sub + 1) * P], pt)
                return ffi, ffiT

            def emit_ffn_mm(st, ffi, ffiT, psH, psF):
                """FFN matmuls + residual + output for stage st."""
                hT = [fp.tile([P, 256], BF16, tag=f"hT{f}", bufs=2,
                              name=f"hT{f}") for f in range(FFC)]
                pf = [psF.tile([P, 512], F32, tag="psF", name="psF")
                      for _ in range(4)]

                def mm2(fk):
                    for sub in range(2):
                        for half in range(2):
                            mm(pf[sub * 2 + half],
                               hT[fk][:, sub * P:(sub + 1) * P],
                               w2_sb[fk][:, half * 512:(half + 1) * 512],
                               start=(fk == 0), stop=(fk == FFC - 1))

                # pipelined: mm2(fk-1) is emitted after mm1(fk) so the PE
                # never waits on the DVE relu of the chunk it just computed.
                for fk in range(FFC):
                    ps = psH.tile([P, 256], F32, tag="psH", name="psH")
                    for k in range(DCH):
                        mm(ps, w1_sb[k][:, fk * P:(fk + 1) * P], ffiT[k],
                           start=(k == 0), stop=(k == DCH - 1))
                    # relu on ACT (idle during FFN); DVE keeps LN + residual
                    nc.scalar.activation(hT[fk], ps, AF.Relu,
                                         bias=b1_col[:, fk:fk + 1])
                    if fk > 0:
                        mm2(fk - 1)
                mm2(FFC - 1)
                for sub in range(2):
                    o = lnp.tile([P, D], F32, tag="o", bufs=2, name="o")
                    for half in range(2):
                        sl = slice(half * 512, (half + 1) * 512)
                        nc.vector.tensor_add(o[:, sl], pf[sub * 2 + half],
                                             ffi[sub][:, sl])
                        if not b2_zero:
                            nc.vector.tensor_add(o[:, sl], o[:, sl],
                                                 b2_b[:, sl])
                        nc.sync.dma_start(
                            out[st * 256 + sub * P:st * 256 + (sub + 1) * P,
                                sl], o[:, sl])

            _mk = lambda lbl: PHASE_MARKS.append((lbl, nc.next_id()))
            with tc.tile_pool(name="psA", bufs=2, space="PSUM") as psA:
                # phase order: attn10 (batch 0, stage 1 — no new inputs) runs
                # before proj1/attn01 so the PE fills the window where batch
                # 1's x stream is still in flight on the serialized DMA
                # engines; both stage-0 phases precede stage 1 so A2A#0
                # fires as early as possible.  Once the projection psum pool
                # closes, its banks deepen psS for the last two phases (the
                # scores pipeline is otherwise throttled by exp latency).
                with (
                    tc.tile_pool(name="acc", bufs=2, space="PSUM") as acc,
                    tc.tile_pool(name="psS", bufs=2, space="PSUM") as psS,
                ):
                    _mk("proj0")

                    def mid0():
                        _mk("attn00")
                        emit_attn(0, 0, psS, psA)
                        _mk("attn10r")

                    emit_proj(0, acc, mid=mid0)
                    _mk("attn10")
                    emit_attn(1, 0, psS, psA)
                    _mk("proj1")
                    emit_proj(1, acc)
                    load_ffn_weights()
                with tc.tile_pool(name="psSb", bufs=3, space="PSUM") as psSb:
                    _mk("attn01")
                    emit_attn(0, 1, psSb, psA)
                    nc.gpsimd.collective_compute(
                        "AllToAll", OP.bypass, replica_groups=REP,
                        ins=[inb[0].opt()], outs=[outb[0].opt()])
                    _mk("attn11")
                    emit_attn(1, 1, psSb, psA)

            with (
                tc.tile_pool(name="psH", bufs=2, space="PSUM") as psH,
                tc.tile_pool(name="psF", bufs=4, space="PSUM") as psF,
                tc.tile_pool(name="psTr", bufs=2, space="PSUM") as psTr,
            ):
                # scheduler fence: without it the ffn PE instructions get
                # hoisted into the attention stream where they head-of-line
                # block the PE on the collective+LN dependency
                _mk("ffn0")
                tc.no_sync_barrier()
                ln0 = emit_ln(0, psTr)
                emit_ffn_mm(0, *ln0, psH, psF)
                nc.gpsimd.collective_compute(
                    "AllToAll", OP.bypass, replica_groups=REP,
                    ins=[inb[1].opt()], outs=[outb[1].opt()])
                _mk("ffn1")
                tc.no_sync_barrier()
                ln1 = emit_ln(1, psTr)
                emit_ffn_mm(1, *ln1, psH, psF)

    nc.compile()
    return nc


def kernel(**inputs) -> np.ndarray:
    import ml_dtypes
    BF = ml_dtypes.bfloat16
    f32 = lambda a: np.asarray(a, dtype=np.float32)
    query, key, value = f32(inputs["query"]), f32(inputs["key"]), f32(inputs["value"])
    scale = 1.0 / np.sqrt(np.float32(DK))
    wq_f = f32(inputs["Wq"]) * scale
    bq_f = f32(inputs["bq"]) * scale
    wk_f, bk_f = f32(inputs["Wk"]), f32(inputs["bk"])
    wv_f, bv_f = f32(inputs["Wv"]), f32(inputs["bv"])
    w1 = np.ascontiguousarray(f32(inputs["W1"]).astype(BF))
    b1 = f32(inputs["b1"])
    w2 = np.ascontiguousarray(f32(inputs["W2"]).astype(BF))
    b2 = f32(inputs["b2"])
    ln_g, ln_b = f32(inputs["ln_g"]), f32(inputs["ln_b"])

    ln_affine = not (np.all(ln_g == 1.0) and np.all(ln_b == 0.0))
    nc = build_program(ln_affine=ln_affine, b2_zero=not b2.any())

    # stage-major token permutation for the query side: position
    # (s*4 + d)*256 + i  <-  natural token d*512 + s*256 + i
    order = np.concatenate([
        np.arange(d * 512 + s * 256, d * 512 + s * 256 + 256)
        for s in range(NST) for d in range(4)])
    xqT = np.ascontiguousarray(np.concatenate(
        [query[b].T[:, order] for b in range(B)], axis=1).astype(BF))
    xkT = np.ascontiguousarray(np.concatenate(
        [key[b].T for b in range(B)], axis=1).astype(BF))
    xvT = np.ascontiguousarray(np.concatenate(
        [value[b].T for b in range(B)], axis=1).astype(BF))

    shared = dict(xqT=xqT, xkT=xkT, xvT=xvT, w1=w1, w2=w2, b1=b1, b2=b2,
                  ln_g=ln_g, ln_b=ln_b)
    in_maps = []
    for c in range(N_CORES):
        cols = slice(P * c, P * (c + 1))
        in_maps.append(dict(
            wq=np.ascontiguousarray(wq_f[:, cols].astype(BF)),
            wk=np.ascontiguousarray(wk_f[:, cols].astype(BF)),
            wv=np.ascontiguousarray(wv_f[:, cols].astype(BF)),
            bq=np.ascontiguousarray(bq_f[cols]),
            bk=np.ascontiguousarray(bk_f[cols]),
            bv=np.ascontiguousarray(bv_f[cols]),
            **shared,
        ))

    res = run_bass_kernel_spmd(nc, in_maps, list(range(N_CORES)))
    out = np.empty((B, S, D), dtype=np.float32)
    for c in range(N_CORES):
        b = c // 4
        t0 = (c % 4) * TOK
        out[b, t0:t0 + TOK, :] = res.results[c]["out"]
    return out
